# revision 27
# baseline (speedup 1.0000x reference)
"""Trainium2 Bass kernel for NonLocalBlock (B=4, C=128, H=W=64).

Sharding: 8 cores = 2 cores per batch sample; each core computes attention for
its half of the spatial tokens (2048 queries) against all 4096 keys of its
sample. Everything (the [Nq, N] attention matrix included) stays in SBUF.

Math simplifications (all exact up to float rounding):
  - phi_b adds a per-query constant to the logits -> softmax-invariant -> drop.
  - g_b and w_b become per-channel constants in w_y -> cancelled exactly by
    train-mode BatchNorm -> drop.
  - exp() without max subtraction: |logits| < ~65, safe in fp32.
Matmuls run in fp32r (TF32-like, ~11 mantissa bits, 4x faster than fp32 on
the PE); x and weights are pre-rounded host-side and declared fp32r.

Per-core dataflow:
  x arrives query-half-first permuted [128, 4096] (key order permutation is
  harmless: phi/g share it).
  theta = wq^T x[:, :2048] + tb  [64, 2048]; phi = wk^T x  [64, 4096]
  gT = x^T wv with a ones column appended    [128, 32, 65]
  per 512-query chunk (software-pipelined, fT one group ahead):
    fT[m, q] = phi_m^T theta_q     (PE -> 3-bank PSUM groups)
    ef = exp(fT)                   (ScalarE, one op per 3 banks)
    acc[65, 512] += gT_m^T ef_m    (PE; row 64 = softmax denominator)
    y = acc[:64] * bcast(1/acc[64]); wy = wo^T y; bn_stats per chunk
  BN: AllGather of per-core bn_stats -> bn_aggr -> scale/shift -> apply + x
"""

import sys

if "/opt/trn_rl_repo" not in sys.path:
    sys.path.insert(0, "/opt/trn_rl_repo")

import numpy as np

import concourse.bass as bass
import concourse.mybir as mybir
import concourse.tile as tile
from concourse import bacc
from concourse import bass_utils

NCORES = 8
P = 128          # channels C
CI = 64          # inter channels
N = 4096         # spatial tokens per sample
NQ = 2048        # queries per core
QCHUNK = 512     # queries per attention chunk
MCHUNK = 128     # keys per fT matmul (partition dim of fT)
NMC = N // MCHUNK          # 32 m-chunks
GROUP = 3                  # m-chunks per PSUM group / exp op
EPS = 1e-5
FP32 = mybir.dt.float32
FP32R = mybir.dt.float32r


def _build(model_only=False):
    nc = bacc.Bacc("TRN2", target_bir_lowering=False, debug=False,
                   num_devices=1 if model_only else NCORES)

    xp = nc.dram_tensor("xp", [P, N], FP32R, kind="ExternalInput").ap()
    wq = nc.dram_tensor("wq", [P, CI], FP32R, kind="ExternalInput").ap()
    wk = nc.dram_tensor("wk", [P, CI], FP32R, kind="ExternalInput").ap()
    wv = nc.dram_tensor("wv", [P, CI], FP32R, kind="ExternalInput").ap()
    wo = nc.dram_tensor("wo", [CI, P], FP32R, kind="ExternalInput").ap()
    tb = nc.dram_tensor("tb", [CI, 1], FP32, kind="ExternalInput").ap()
    bng = nc.dram_tensor("bng", [P, 1], FP32, kind="ExternalInput").ap()
    bnb = nc.dram_tensor("bnb", [P, 1], FP32, kind="ExternalInput").ap()
    out = nc.dram_tensor("out", [P, NQ], FP32, kind="ExternalOutput").ap()

    Exp = mybir.ActivationFunctionType.Exp

    with tile.TileContext(nc) as tc:
        with tc.tile_pool(name="cst", bufs=1) as cst, \
             tc.tile_pool(name="big", bufs=1) as big, \
             tc.tile_pool(name="efp", bufs=3) as efp, \
             tc.tile_pool(name="small", bufs=2) as small, \
             tc.tile_pool(name="ftp", bufs=2, space="PSUM") as ftp, \
             tc.tile_pool(name="yp", bufs=1, space="PSUM") as yp, \
             tc.tile_pool(name="mp", bufs=1, space="PSUM") as mp, \
             tc.tile_pool(name="dram", bufs=1, space="DRAM") as dram:

            # ---- loads: weights first (tiny), x spread over 3 DGE queues
            wq_sb = cst.tile([P, CI], FP32R)
            nc.sync.dma_start(wq_sb, wq)
            wk_sb = cst.tile([P, CI], FP32R)
            nc.scalar.dma_start(wk_sb, wk)
            wv_sb = cst.tile([P, CI], FP32R)
            nc.gpsimd.dma_start(wv_sb, wv)
            wo_sb = cst.tile([CI, P], FP32R)
            nc.gpsimd.dma_start(wo_sb, wo)
            tb_sb = cst.tile([CI, 1], FP32)
            nc.sync.dma_start(tb_sb, tb)
            bng_sb = cst.tile([P, 1], FP32)
            nc.sync.dma_start(bng_sb, bng)
            bnb_sb = cst.tile([P, 1], FP32)
            nc.scalar.dma_start(bnb_sb, bnb)
            x_sb = big.tile([P, N], FP32R)
            dma_engines = [nc.sync, nc.scalar, nc.gpsimd]
            for j in range(8):
                s = bass.ts(j, 512)
                dma_engines[j % 3].dma_start(x_sb[:, s], xp[:, s])
            epsb = cst.tile([P, 1], FP32)
            nc.vector.memset(epsb, EPS)
            ones1 = cst.tile([1, P], FP32)
            nc.vector.memset(ones1, 1.0)

            # ---- projections: per 512-col chunk, one 3-bank PSUM tile
            # holds theta/phi/gT sections; 2-3 DVE moves per chunk ----
            th_sb = big.tile([CI, NQ], FP32R)
            ph_sb = big.tile([CI, N], FP32R)
            gt_sb = big.tile([P, NMC, CI + 1], FP32R)
            nc.vector.tensor_scalar(gt_sb[:, :, CI],
                                    x_sb[:, 0:NMC].bitcast(FP32),
                                    scalar1=0.0, scalar2=1.0,
                                    op0=mybir.AluOpType.mult,
                                    op1=mybir.AluOpType.add)
            def emit_proj(j):
                s = bass.ts(j, 512)
                pt = ftp.tile([P, GROUP * QCHUNK], FP32, tag="ft",
                              name=f"proj{j}")
                if j < NQ // 512:
                    nc.tensor.matmul(pt[0:CI, 0:512], lhsT=wq_sb,
                                     rhs=x_sb[:, s], start=True, stop=True)
                nc.tensor.matmul(pt[0:CI, 512:1024], lhsT=wk_sb,
                                 rhs=x_sb[:, s], start=True, stop=True)
                gsec = pt[:, 1024:1280].rearrange("p (a b) -> p a b", b=CI)
                for k in range(4):
                    mi = 4 * j + k
                    nc.tensor.matmul(gsec[:, k, :],
                                     lhsT=x_sb[:, bass.ts(mi, MCHUNK)],
                                     rhs=wv_sb, start=True, stop=True)
                if j < NQ // 512:
                    nc.vector.tensor_scalar_add(th_sb[:, s], pt[0:CI, 0:512],
                                                tb_sb)
                nc.vector.tensor_copy(ph_sb[:, s], pt[0:CI, 512:1024])
                nc.vector.tensor_copy(gt_sb[:, 4 * j:4 * j + 4, 0:CI], gsec)

            # ---- attention (software-pipelined) ----
            wy_sb = big.tile([P, NQ], FP32)
            nqc = NQ // QCHUNK
            stats_sb = big.tile([P, nqc, 6], FP32)
            groups = [list(range(g, min(g + GROUP, NMC)))
                      for g in range(0, NMC, GROUP)]
            stream = [(qc, gi) for qc in range(nqc)
                      for gi in range(len(groups))]
            yps_by_qc = {}

            def emit_ft(qc, gi):
                chunks = groups[gi]
                ft = ftp.tile([P, GROUP * QCHUNK], FP32, tag="ft")
                for j, mi in enumerate(chunks):
                    nc.tensor.matmul(ft[:, bass.ts(j, QCHUNK)],
                                     lhsT=ph_sb[:, bass.ts(mi, MCHUNK)],
                                     rhs=th_sb[:, bass.ts(qc, QCHUNK)],
                                     start=True, stop=True)
                return ft

            def emit_exp_y(qc, gi, ft):
                chunks = groups[gi]
                w = QCHUNK * len(chunks)
                if qc not in yps_by_qc:
                    yps_by_qc[qc] = yp.tile([CI + 1, QCHUNK], FP32,
                                            tag="yps", name=f"yps{qc}")
                yps = yps_by_qc[qc]
                ef = efp.tile([P, GROUP * QCHUNK], FP32R, tag="ef")
                nc.scalar.activation(ef[:, :w], ft[:, :w], Exp)
                for j, mi in enumerate(chunks):
                    nc.tensor.matmul(yps, lhsT=gt_sb[:, mi, :],
                                     rhs=ef[:, bass.ts(j, QCHUNK)],
                                     start=(mi == 0), stop=(mi == NMC - 1))

            def emit_tail(qc):
                qs = bass.ts(qc, QCHUNK)
                yps = yps_by_qc.pop(qc)
                sums_sb = small.tile([1, QCHUNK], FP32, tag="sums")
                nc.vector.tensor_copy(sums_sb, yps[CI:CI + 1, :])
                rec_sb = small.tile([1, QCHUNK], FP32, tag="rec")
                nc.vector.reciprocal_approx_fast(rec_sb, sums_sb)
                bc = mp.tile([P, QCHUNK], FP32, tag="mp")
                nc.tensor.matmul(bc, lhsT=ones1, rhs=rec_sb,
                                 start=True, stop=True)
                yun_sb = small.tile([CI, QCHUNK], FP32, tag="yun")
                nc.vector.tensor_copy(yun_sb, yps[0:CI, :])
                yn_sb = small.tile([CI, QCHUNK], FP32R, tag="yn")
                nc.vector.tensor_mul(yn_sb, yun_sb, bc[0:CI, :])
                wps = mp.tile([P, QCHUNK], FP32, tag="mp")
                nc.tensor.matmul(wps, lhsT=wo_sb, rhs=yn_sb,
                                 start=True, stop=True)
                nc.vector.tensor_copy(wy_sb[:, qs], wps)
                nc.vector.bn_stats(stats_sb[:, qc, :], wy_sb[:, qs])

            n_proj = 0

            def need_proj(jreq):
                nonlocal n_proj
                while n_proj <= jreq:
                    emit_proj(n_proj)
                    n_proj += 1

            pending = None
            for qc, gi in stream:
                if qc == 0:
                    need_proj(min(GROUP * gi + GROUP - 1, NMC - 1) // 4)
                ft = emit_ft(qc, gi)
                if pending is not None:
                    pqc, pgi, pft = pending
                    emit_exp_y(pqc, pgi, pft)
                    if pgi == len(groups) - 1:
                        emit_tail(pqc)
                pending = (qc, gi, ft)
            need_proj(7)
            pqc, pgi, pft = pending
            emit_exp_y(pqc, pgi, pft)
            emit_tail(pqc)

            # ---- global BN stats via AllGather ----
            cc_in = dram.tile([P, nqc * 6], FP32)
            nc.sync.dma_start(cc_in,
                              stats_sb[:].rearrange("p a b -> p (a b)"))
            cc_out = dram.tile([NCORES, P, nqc * 6], FP32)
            if model_only:
                for r in range(NCORES):
                    nc.sync.dma_start(cc_out[r], cc_in)
            else:
                nc.gpsimd.collective_compute(
                    "AllGather",
                    mybir.AluOpType.bypass,
                    replica_groups=[list(range(NCORES))],
                    ins=[cc_in[:].opt()],
                    outs=[cc_out[:].opt()],
                )
            gst = big.tile([P, NCORES, nqc * 6], FP32)
            nc.sync.dma_start(gst, cc_out[:].rearrange("r p j -> p r j"))
            gmv = small.tile([P, 2], FP32, tag="gmv")
            nc.vector.bn_aggr(gmv, gst[:].rearrange("p r (a b) -> p (r a) b",
                                                    b=6))
            # rsqrt(var+eps) = exp(-0.5*ln(var+eps)) — stays in the exp/ln
            # ACT table set, avoiding a ~2.7us table switch
            lnv = small.tile([P, 1], FP32, tag="lnv")
            nc.scalar.activation(lnv, gmv[:, 1:2],
                                 mybir.ActivationFunctionType.Ln, bias=epsb)
            inv = small.tile([P, 1], FP32, tag="inv")
            nc.scalar.activation(inv, lnv, Exp, scale=-0.5)
            scl = small.tile([P, 1], FP32, tag="scl")
            nc.vector.tensor_mul(scl, bng_sb, inv)
            msc = small.tile([P, 1], FP32, tag="msc")
            nc.vector.tensor_mul(msc, gmv[:, 0:1], scl)
            sh = small.tile([P, 1], FP32, tag="sh")
            nc.vector.tensor_sub(sh, bnb_sb, msc)

            # ---- apply BN + residual (chunked so DMA overlaps DVE) ----
            out_sb = big.tile([P, NQ], FP32)
            xs_sb = big.tile([P, NQ], FP32)
            for j in range(4):
                s = bass.ts(j, 512)
                nc.vector.tensor_scalar_add(xs_sb[:, s],
                                            x_sb[:, s].bitcast(FP32), sh)
                nc.vector.scalar_tensor_tensor(out_sb[:, s], wy_sb[:, s],
                                               scl, xs_sb[:, s],
                                               op0=mybir.AluOpType.mult,
                                               op1=mybir.AluOpType.add)
                dma_engines[j % 3].dma_start(out[:, s], out_sb[:, s])

    nc.compile()
    return nc


_NC = None


def _get_nc():
    global _NC
    if _NC is None:
        _NC = _build()
    return _NC


def _round_tf32(a):
    """Round-to-nearest-even keeping 11 mantissa bits (fp32r pre-rounding)."""
    u = np.ascontiguousarray(np.asarray(a, np.float32)).view(np.uint32).copy()
    u += np.uint32(0x7FF) + ((u >> np.uint32(12)) & np.uint32(1))
    u &= np.uint32(0xFFFFF000)
    return u.view(np.float32)


def kernel(x, g_w, g_b, theta_w, theta_b, phi_w, phi_b, w_w, w_b,
           bn_gamma, bn_beta, _trace=False):
    x = np.ascontiguousarray(np.asarray(x, dtype=np.float32))
    B = x.shape[0]
    xf = x.reshape(B, P, N)

    f32 = lambda a: np.ascontiguousarray(np.asarray(a, dtype=np.float32))
    common = {
        "wq": _round_tf32(np.asarray(theta_w).T),
        "wk": _round_tf32(np.asarray(phi_w).T),
        "wv": _round_tf32(np.asarray(g_w).T),
        "wo": _round_tf32(np.asarray(w_w).T),
        "tb": f32(np.asarray(theta_b)[:, None]),
        "bng": f32(np.asarray(bn_gamma)[:, None]),
        "bnb": f32(np.asarray(bn_beta)[:, None]),
    }

    in_maps = []
    for core in range(NCORES):
        b, h = core // 2, core % 2
        xs = xf[b]
        xpc = np.concatenate(
            [xs[:, h * NQ:(h + 1) * NQ], xs[:, (1 - h) * NQ:(2 - h) * NQ]],
            axis=1)
        in_maps.append({"xp": _round_tf32(xpc), **common})

    global _last_in_maps
    _last_in_maps = in_maps
    nc = _get_nc()
    res = bass_utils.run_bass_kernel_spmd(nc, in_maps,
                                          core_ids=list(range(NCORES)),
                                          trace=_trace)
    if _trace:
        global last_exec_time_ns, last_results
        last_exec_time_ns = res.exec_time_ns
        last_results = res

    outf = np.empty((B, P, N), dtype=np.float32)
    for core in range(NCORES):
        b, h = core // 2, core % 2
        outf[b, :, h * NQ:(h + 1) * NQ] = res.results[core]["out"]
    return outf.reshape(x.shape)


# revision 33
# speedup vs baseline: 1.9915x; 1.9915x over previous
"""Trainium2 Bass kernel for NonLocalBlock (B=4, C=128, H=W=64).

Sharding: 8 cores = 2 cores per batch sample; each core computes attention for
its half of the spatial tokens (2048 queries) against all 4096 keys of its
sample. Everything (the [Nq, N] attention matrix included) stays in SBUF.

Math simplifications (all exact up to float rounding):
  - phi_b adds a per-query constant to the logits -> softmax-invariant -> drop.
  - g_b and w_b become per-channel constants in w_y -> cancelled exactly by
    train-mode BatchNorm -> drop.
  - exp() without max subtraction: |logits| < ~65, safe in fp32.
Matmuls run in fp32r (TF32-like, ~11 mantissa bits, 4x faster than fp32 on
the PE); x and weights are pre-rounded host-side and declared fp32r.

Per-core dataflow:
  x arrives query-half-first permuted [128, 4096] (key order permutation is
  harmless: phi/g share it).
  theta = wq^T x[:, :2048] + tb  [64, 2048]; phi = wk^T x  [64, 4096]
  gT = x^T wv with a ones column appended    [128, 32, 65]
  per 512-query chunk (software-pipelined, fT one group ahead):
    fT[m, q] = phi_m^T theta_q     (PE -> 3-bank PSUM groups)
    ef = exp(fT)                   (ScalarE, one op per 3 banks)
    acc[65, 512] += gT_m^T ef_m    (PE; row 64 = softmax denominator)
    y = acc[:64] * bcast(1/acc[64]); wy = wo^T y; bn_stats per chunk
  BN: AllGather of per-core bn_stats -> bn_aggr -> scale/shift -> apply + x
"""

import sys

if "/opt/trn_rl_repo" not in sys.path:
    sys.path.insert(0, "/opt/trn_rl_repo")

import numpy as np

import concourse.bass as bass
import concourse.mybir as mybir
import concourse.tile as tile
from concourse import bacc
from concourse import bass_utils

NCORES = 8
P = 128          # channels C
CI = 64          # inter channels
N = 4096         # spatial tokens per sample
NQ = 2048        # queries per core
QCHUNK = 512     # queries per attention chunk
MCHUNK = 128     # keys per fT matmul (partition dim of fT)
NMC = N // MCHUNK          # 32 m-chunks
GROUP = 2                  # m-chunks per PSUM group / exp op
EPS = 1e-5
FP32 = mybir.dt.float32
FP32R = mybir.dt.float32r


def _build(model_only=False):
    nc = bacc.Bacc("TRN2", target_bir_lowering=False, debug=False,
                   num_devices=1 if model_only else NCORES)

    xp = nc.dram_tensor("xp", [P, N], FP32R, kind="ExternalInput").ap()
    wq = nc.dram_tensor("wq", [P, CI], FP32R, kind="ExternalInput").ap()
    wk = nc.dram_tensor("wk", [P, CI], FP32R, kind="ExternalInput").ap()
    wv = nc.dram_tensor("wv", [P, CI], FP32R, kind="ExternalInput").ap()
    wo = nc.dram_tensor("wo", [CI, P], FP32R, kind="ExternalInput").ap()
    tb = nc.dram_tensor("tb", [CI, 1], FP32, kind="ExternalInput").ap()
    bng = nc.dram_tensor("bng", [P, 1], FP32, kind="ExternalInput").ap()
    bnb = nc.dram_tensor("bnb", [P, 1], FP32, kind="ExternalInput").ap()
    out = nc.dram_tensor("out", [P, NQ], FP32, kind="ExternalOutput").ap()

    Exp = mybir.ActivationFunctionType.Exp

    with tile.TileContext(nc) as tc:
        with tc.tile_pool(name="cst", bufs=1) as cst, \
             tc.tile_pool(name="big", bufs=1) as big, \
             tc.tile_pool(name="efp", bufs=4) as efp, \
             tc.tile_pool(name="small", bufs=2) as small, \
             tc.tile_pool(name="ftp", bufs=3, space="PSUM") as ftp, \
             tc.tile_pool(name="yp", bufs=1, space="PSUM") as yp, \
             tc.tile_pool(name="mp", bufs=1, space="PSUM") as mp, \
             tc.tile_pool(name="dram", bufs=1, space="DRAM") as dram:

            # ---- loads: weights first (tiny), x spread over 3 DGE queues
            wq_sb = cst.tile([P, CI], FP32R)
            nc.sync.dma_start(wq_sb, wq)
            wk_sb = cst.tile([P, CI], FP32R)
            nc.scalar.dma_start(wk_sb, wk)
            wv_sb = cst.tile([P, CI], FP32R)
            nc.gpsimd.dma_start(wv_sb, wv)
            wo_sb = cst.tile([CI, P], FP32R)
            nc.gpsimd.dma_start(wo_sb, wo)
            tb_sb = cst.tile([CI, 1], FP32)
            nc.sync.dma_start(tb_sb, tb)
            bng_sb = cst.tile([P, 1], FP32)
            nc.sync.dma_start(bng_sb, bng)
            bnb_sb = cst.tile([P, 1], FP32)
            nc.scalar.dma_start(bnb_sb, bnb)
            x_sb = big.tile([P, N], FP32R)
            dma_engines = [nc.sync, nc.scalar, nc.gpsimd]
            for j in range(8):
                s = bass.ts(j, 512)
                dma_engines[j % 3].dma_start(x_sb[:, s], xp[:, s])
            epsb = cst.tile([P, 1], FP32)
            nc.vector.memset(epsb, EPS)
            ones1 = cst.tile([1, P], FP32)
            nc.vector.memset(ones1, 1.0)

            # ---- projections: per 512-col chunk, one 3-bank PSUM tile
            # holds theta/phi/gT sections; 2-3 DVE moves per chunk ----
            th_sb = big.tile([CI, NQ], FP32R)
            ph_sb = big.tile([CI, N], FP32R)
            gt_sb = big.tile([P, NMC, CI + 1], FP32R)
            nc.vector.tensor_scalar(gt_sb[:, :, CI],
                                    x_sb[:, 0:NMC].bitcast(FP32),
                                    scalar1=0.0, scalar2=1.0,
                                    op0=mybir.AluOpType.mult,
                                    op1=mybir.AluOpType.add)
            def emit_proj(j):
                s = bass.ts(j, 512)
                pt = ftp.tile([P, 2 * QCHUNK], FP32, tag="ft",
                              name=f"proj{j}")
                if j < NQ // 512:
                    nc.tensor.matmul(pt[0:CI, 0:512], lhsT=wq_sb,
                                     rhs=x_sb[:, s], start=True, stop=True)
                nc.tensor.matmul(pt[0:CI, 512:1024], lhsT=wk_sb,
                                 rhs=x_sb[:, s], start=True, stop=True)
                gt = ftp.tile([P, 4, CI], FP32, tag="ft", name=f"projg{j}")
                for k in range(4):
                    mi = 4 * j + k
                    nc.tensor.matmul(gt[:, k, :],
                                     lhsT=x_sb[:, bass.ts(mi, MCHUNK)],
                                     rhs=wv_sb, start=True, stop=True)
                if j < NQ // 512:
                    nc.vector.tensor_scalar_add(th_sb[:, s], pt[0:CI, 0:512],
                                                tb_sb)
                nc.vector.tensor_copy(ph_sb[:, s], pt[0:CI, 512:1024])
                nc.vector.tensor_copy(gt_sb[:, 4 * j:4 * j + 4, 0:CI], gt)

            # ---- attention (software-pipelined) ----
            wy_sb = big.tile([P, NQ], FP32)
            nqc = NQ // QCHUNK
            stats_sb = big.tile([P, nqc, 6], FP32)
            groups = [list(range(g, min(g + GROUP, NMC)))
                      for g in range(0, NMC, GROUP)]
            stream = [(qc, gi) for qc in range(nqc)
                      for gi in range(len(groups))]
            yps_by_qc = {}

            def emit_ft(qc, gi):
                chunks = groups[gi]
                ft = ftp.tile([P, GROUP * QCHUNK], FP32, tag="ft")
                for j, mi in enumerate(chunks):
                    nc.tensor.matmul(ft[:, bass.ts(j, QCHUNK)],
                                     lhsT=ph_sb[:, bass.ts(mi, MCHUNK)],
                                     rhs=th_sb[:, bass.ts(qc, QCHUNK)],
                                     start=True, stop=True)
                return ft

            def emit_exp_y(qc, gi, ft):
                chunks = groups[gi]
                w = QCHUNK * len(chunks)
                if qc not in yps_by_qc:
                    yps_by_qc[qc] = yp.tile([CI + 1, QCHUNK], FP32,
                                            tag="yps", name=f"yps{qc}")
                yps = yps_by_qc[qc]
                ef = efp.tile([P, GROUP * QCHUNK], FP32R, tag="ef")
                nc.scalar.activation(ef[:, :w], ft[:, :w], Exp)
                for j, mi in enumerate(chunks):
                    nc.tensor.matmul(yps, lhsT=gt_sb[:, mi, :],
                                     rhs=ef[:, bass.ts(j, QCHUNK)],
                                     start=(mi == 0), stop=(mi == NMC - 1))

            def emit_tail(qc):
                qs = bass.ts(qc, QCHUNK)
                yps = yps_by_qc.pop(qc)
                sums_sb = small.tile([1, QCHUNK], FP32, tag="sums")
                nc.vector.tensor_copy(sums_sb, yps[CI:CI + 1, :])
                rec_sb = small.tile([1, QCHUNK], FP32, tag="rec")
                nc.vector.reciprocal_approx_fast(rec_sb, sums_sb)
                bc = mp.tile([P, QCHUNK], FP32, tag="mp")
                nc.tensor.matmul(bc, lhsT=ones1, rhs=rec_sb,
                                 start=True, stop=True)
                yun_sb = small.tile([CI, QCHUNK], FP32, tag="yun")
                nc.vector.tensor_copy(yun_sb, yps[0:CI, :])
                yn_sb = small.tile([CI, QCHUNK], FP32R, tag="yn")
                nc.vector.tensor_mul(yn_sb, yun_sb, bc[0:CI, :])
                wps = mp.tile([P, QCHUNK], FP32, tag="mp")
                nc.tensor.matmul(wps, lhsT=wo_sb, rhs=yn_sb,
                                 start=True, stop=True)
                nc.vector.tensor_copy(wy_sb[:, qs], wps)
                nc.vector.bn_stats(stats_sb[:, qc, :], wy_sb[:, qs])

            n_proj = 0

            def need_proj(jreq):
                nonlocal n_proj
                while n_proj <= jreq:
                    emit_proj(n_proj)
                    n_proj += 1

            pending = None
            for qc, gi in stream:
                if qc == 0:
                    need_proj(min(GROUP * gi + GROUP - 1, NMC - 1) // 4)
                ft = emit_ft(qc, gi)
                if pending is not None:
                    pqc, pgi, pft = pending
                    emit_exp_y(pqc, pgi, pft)
                    if pgi == len(groups) - 1:
                        emit_tail(pqc)
                pending = (qc, gi, ft)
            need_proj(7)
            pqc, pgi, pft = pending
            emit_exp_y(pqc, pgi, pft)
            emit_tail(pqc)

            # ---- global BN stats via AllGather ----
            cc_in = dram.tile([P, nqc * 6], FP32)
            nc.sync.dma_start(cc_in,
                              stats_sb[:].rearrange("p a b -> p (a b)"))
            cc_out = dram.tile([NCORES, P, nqc * 6], FP32)
            if model_only:
                for r in range(NCORES):
                    nc.sync.dma_start(cc_out[r], cc_in)
            else:
                nc.gpsimd.collective_compute(
                    "AllGather",
                    mybir.AluOpType.bypass,
                    replica_groups=[list(range(NCORES))],
                    ins=[cc_in[:].opt()],
                    outs=[cc_out[:].opt()],
                )
            gst = big.tile([P, NCORES, nqc * 6], FP32)
            nc.sync.dma_start(gst, cc_out[:].rearrange("r p j -> p r j"))
            gmv = small.tile([P, 2], FP32, tag="gmv")
            nc.vector.bn_aggr(gmv, gst[:].rearrange("p r (a b) -> p (r a) b",
                                                    b=6))
            # rsqrt(var+eps) = exp(-0.5*ln(var+eps)) — stays in the exp/ln
            # ACT table set, avoiding a ~2.7us table switch
            lnv = small.tile([P, 1], FP32, tag="lnv")
            nc.scalar.activation(lnv, gmv[:, 1:2],
                                 mybir.ActivationFunctionType.Ln, bias=epsb)
            inv = small.tile([P, 1], FP32, tag="inv")
            nc.scalar.activation(inv, lnv, Exp, scale=-0.5)
            scl = small.tile([P, 1], FP32, tag="scl")
            nc.vector.tensor_mul(scl, bng_sb, inv)
            msc = small.tile([P, 1], FP32, tag="msc")
            nc.vector.tensor_mul(msc, gmv[:, 0:1], scl)
            sh = small.tile([P, 1], FP32, tag="sh")
            nc.vector.tensor_sub(sh, bnb_sb, msc)

            # ---- apply BN + residual (chunked so DMA overlaps DVE) ----
            out_sb = big.tile([P, NQ], FP32)
            xs_sb = big.tile([P, NQ], FP32)
            for j in range(4):
                s = bass.ts(j, 512)
                nc.vector.tensor_scalar_add(xs_sb[:, s],
                                            x_sb[:, s].bitcast(FP32), sh)
                nc.vector.scalar_tensor_tensor(out_sb[:, s], wy_sb[:, s],
                                               scl, xs_sb[:, s],
                                               op0=mybir.AluOpType.mult,
                                               op1=mybir.AluOpType.add)
                dma_engines[j % 3].dma_start(out[:, s], out_sb[:, s])

    nc.compile()
    return nc


_NC = None


def _get_nc():
    global _NC
    if _NC is None:
        _NC = _build()
    return _NC


def _round_tf32(a):
    """Round-to-nearest-even keeping 11 mantissa bits (fp32r pre-rounding)."""
    u = np.ascontiguousarray(np.asarray(a, np.float32)).view(np.uint32).copy()
    u += np.uint32(0x7FF) + ((u >> np.uint32(12)) & np.uint32(1))
    u &= np.uint32(0xFFFFF000)
    return u.view(np.float32)


def kernel(x, g_w, g_b, theta_w, theta_b, phi_w, phi_b, w_w, w_b,
           bn_gamma, bn_beta, _trace=False):
    x = np.ascontiguousarray(np.asarray(x, dtype=np.float32))
    B = x.shape[0]
    xf = x.reshape(B, P, N)

    f32 = lambda a: np.ascontiguousarray(np.asarray(a, dtype=np.float32))
    common = {
        "wq": _round_tf32(np.asarray(theta_w).T),
        "wk": _round_tf32(np.asarray(phi_w).T),
        "wv": _round_tf32(np.asarray(g_w).T),
        "wo": _round_tf32(np.asarray(w_w).T),
        "tb": f32(np.asarray(theta_b)[:, None]),
        "bng": f32(np.asarray(bn_gamma)[:, None]),
        "bnb": f32(np.asarray(bn_beta)[:, None]),
    }

    in_maps = []
    for core in range(NCORES):
        b, h = core // 2, core % 2
        xs = xf[b]
        xpc = np.concatenate(
            [xs[:, h * NQ:(h + 1) * NQ], xs[:, (1 - h) * NQ:(2 - h) * NQ]],
            axis=1)
        in_maps.append({"xp": _round_tf32(xpc), **common})

    global _last_in_maps
    _last_in_maps = in_maps
    nc = _get_nc()
    res = bass_utils.run_bass_kernel_spmd(nc, in_maps,
                                          core_ids=list(range(NCORES)),
                                          trace=_trace)
    if _trace:
        global last_exec_time_ns, last_results
        last_exec_time_ns = res.exec_time_ns
        last_results = res

    outf = np.empty((B, P, N), dtype=np.float32)
    for core in range(NCORES):
        b, h = core // 2, core % 2
        outf[b, :, h * NQ:(h + 1) * NQ] = res.results[core]["out"]
    return outf.reshape(x.shape)


# revision 35
# speedup vs baseline: 6.8392x; 3.4341x over previous
"""Trainium2 Bass kernel for NonLocalBlock (B=4, C=128, H=W=64).

Sharding: 8 cores = 2 cores per batch sample; each core computes attention for
its half of the spatial tokens (2048 queries) against all 4096 keys of its
sample. Everything (the [Nq, N] attention matrix included) stays in SBUF.

Math simplifications (all exact up to float rounding):
  - phi_b adds a per-query constant to the logits -> softmax-invariant -> drop.
  - g_b and w_b become per-channel constants in w_y -> cancelled exactly by
    train-mode BatchNorm -> drop.
  - exp() without max subtraction: |logits| < ~65, safe in fp32.
Matmuls run in fp32r (TF32-like, ~11 mantissa bits, 4x faster than fp32 on
the PE); x and weights are pre-rounded host-side and declared fp32r.

Per-core dataflow:
  x arrives query-half-first permuted [128, 4096] (key order permutation is
  harmless: phi/g share it).
  theta = wq^T x[:, :2048] + tb  [64, 2048]; phi = wk^T x  [64, 4096]
  gT = x^T wv with a ones column appended    [128, 32, 65]
  per 512-query chunk (software-pipelined, fT one group ahead):
    fT[m, q] = phi_m^T theta_q     (PE -> 3-bank PSUM groups)
    ef = exp(fT)                   (ScalarE, one op per 3 banks)
    acc[65, 512] += gT_m^T ef_m    (PE; row 64 = softmax denominator)
    y = acc[:64] * bcast(1/acc[64]); wy = wo^T y; bn_stats per chunk
  BN: AllGather of per-core bn_stats -> bn_aggr -> scale/shift -> apply + x
"""

import sys

if "/opt/trn_rl_repo" not in sys.path:
    sys.path.insert(0, "/opt/trn_rl_repo")

import numpy as np

import concourse.bass as bass
import concourse.mybir as mybir
import concourse.tile as tile
from concourse import bacc
from concourse import bass_utils

NCORES = 8
P = 128          # channels C
CI = 64          # inter channels
N = 4096         # spatial tokens per sample
NQ = 2048        # queries per core
QCHUNK = 512     # queries per attention chunk
MCHUNK = 128     # keys per fT matmul (partition dim of fT)
NMC = N // MCHUNK          # 32 m-chunks
GROUP = 2                  # m-chunks per PSUM group / exp op
EPS = 1e-5
FP32 = mybir.dt.float32
FP32R = mybir.dt.float32r


def _build(model_only=False):
    nc = bacc.Bacc("TRN2", target_bir_lowering=False, debug=False,
                   num_devices=1 if model_only else NCORES)

    xp = nc.dram_tensor("xp", [P, N], FP32R, kind="ExternalInput").ap()
    wq = nc.dram_tensor("wq", [P, CI], FP32R, kind="ExternalInput").ap()
    wk = nc.dram_tensor("wk", [P, CI], FP32R, kind="ExternalInput").ap()
    wv = nc.dram_tensor("wv", [P, CI], FP32R, kind="ExternalInput").ap()
    wo = nc.dram_tensor("wo", [CI, P], FP32R, kind="ExternalInput").ap()
    tb = nc.dram_tensor("tb", [CI, 1], FP32, kind="ExternalInput").ap()
    bng = nc.dram_tensor("bng", [P, 1], FP32, kind="ExternalInput").ap()
    bnb = nc.dram_tensor("bnb", [P, 1], FP32, kind="ExternalInput").ap()
    out = nc.dram_tensor("out", [P, NQ], FP32, kind="ExternalOutput").ap()

    Exp = mybir.ActivationFunctionType.Exp

    with tile.TileContext(nc) as tc:
        with tc.tile_pool(name="cst", bufs=1) as cst, \
             tc.tile_pool(name="big", bufs=1) as big, \
             tc.tile_pool(name="efp", bufs=4) as efp, \
             tc.tile_pool(name="small", bufs=2) as small, \
             tc.tile_pool(name="ftp", bufs=3, space="PSUM") as ftp, \
             tc.tile_pool(name="yp", bufs=1, space="PSUM") as yp, \
             tc.tile_pool(name="mp", bufs=1, space="PSUM") as mp, \
             tc.tile_pool(name="dram", bufs=1, space="DRAM") as dram:

            # ---- loads: weights first (tiny), x spread over 3 DGE queues
            wq_sb = cst.tile([P, CI], FP32R)
            nc.sync.dma_start(wq_sb, wq)
            wk_sb = cst.tile([P, CI], FP32R)
            nc.scalar.dma_start(wk_sb, wk)
            wv_sb = cst.tile([P, CI], FP32R)
            nc.gpsimd.dma_start(wv_sb, wv)
            wo_sb = cst.tile([CI, P], FP32R)
            nc.gpsimd.dma_start(wo_sb, wo)
            tb_sb = cst.tile([CI, 1], FP32)
            nc.sync.dma_start(tb_sb, tb)
            bng_sb = cst.tile([P, 1], FP32)
            nc.sync.dma_start(bng_sb, bng)
            bnb_sb = cst.tile([P, 1], FP32)
            nc.scalar.dma_start(bnb_sb, bnb)
            x_sb = big.tile([P, N], FP32R)
            dma_engines = [nc.sync, nc.scalar, nc.gpsimd]
            for j in range(8):
                s = bass.ts(j, 512)
                dma_engines[j % 3].dma_start(x_sb[:, s], xp[:, s])
            epsb = cst.tile([P, 1], FP32)
            nc.vector.memset(epsb, EPS)
            lnwarm = cst.tile([1, 1], FP32)
            nc.scalar.activation(lnwarm, epsb[0:1, :],
                                 mybir.ActivationFunctionType.Ln)
            ones1 = cst.tile([1, P], FP32)
            nc.vector.memset(ones1, 1.0)

            # ---- projections: per 512-col chunk, one 3-bank PSUM tile
            # holds theta/phi/gT sections; 2-3 DVE moves per chunk ----
            th_sb = big.tile([CI, NQ], FP32R)
            ph_sb = big.tile([CI, N], FP32R)
            gt_sb = big.tile([P, NMC, CI + 1], FP32R)
            nc.vector.tensor_scalar(gt_sb[:, :, CI],
                                    x_sb[:, 0:NMC].bitcast(FP32),
                                    scalar1=0.0, scalar2=1.0,
                                    op0=mybir.AluOpType.mult,
                                    op1=mybir.AluOpType.add)
            def emit_proj(j):
                s = bass.ts(j, 512)
                pt = ftp.tile([P, 2 * QCHUNK], FP32, tag="ft",
                              name=f"proj{j}")
                if j < NQ // 512:
                    nc.tensor.matmul(pt[0:CI, 0:512], lhsT=wq_sb,
                                     rhs=x_sb[:, s], start=True, stop=True)
                nc.tensor.matmul(pt[0:CI, 512:1024], lhsT=wk_sb,
                                 rhs=x_sb[:, s], start=True, stop=True)
                gt = ftp.tile([P, 4, CI], FP32, tag="ft", name=f"projg{j}")
                for k in range(4):
                    mi = 4 * j + k
                    nc.tensor.matmul(gt[:, k, :],
                                     lhsT=x_sb[:, bass.ts(mi, MCHUNK)],
                                     rhs=wv_sb, start=True, stop=True)
                if j < NQ // 512:
                    nc.vector.tensor_scalar_add(th_sb[:, s], pt[0:CI, 0:512],
                                                tb_sb)
                nc.vector.tensor_copy(ph_sb[:, s], pt[0:CI, 512:1024])
                nc.vector.tensor_copy(gt_sb[:, 4 * j:4 * j + 4, 0:CI], gt)

            # ---- attention (software-pipelined) ----
            wy_sb = big.tile([P, NQ], FP32)
            nqc = NQ // QCHUNK
            stats_sb = big.tile([P, nqc, 6], FP32)
            groups = [list(range(g, min(g + GROUP, NMC)))
                      for g in range(0, NMC, GROUP)]
            stream = [(qc, gi) for qc in range(nqc)
                      for gi in range(len(groups))]
            yps_by_qc = {}

            def emit_ft(qc, gi):
                chunks = groups[gi]
                ft = ftp.tile([P, GROUP * QCHUNK], FP32, tag="ft")
                for j, mi in enumerate(chunks):
                    nc.tensor.matmul(ft[:, bass.ts(j, QCHUNK)],
                                     lhsT=ph_sb[:, bass.ts(mi, MCHUNK)],
                                     rhs=th_sb[:, bass.ts(qc, QCHUNK)],
                                     start=True, stop=True)
                return ft

            def emit_exp_y(qc, gi, ft):
                chunks = groups[gi]
                w = QCHUNK * len(chunks)
                if qc not in yps_by_qc:
                    yps_by_qc[qc] = yp.tile([CI + 1, QCHUNK], FP32,
                                            tag="yps", name=f"yps{qc}")
                yps = yps_by_qc[qc]
                ef = efp.tile([P, GROUP * QCHUNK], FP32R, tag="ef")
                nc.scalar.activation(ef[:, :w], ft[:, :w], Exp)
                for j, mi in enumerate(chunks):
                    nc.tensor.matmul(yps, lhsT=gt_sb[:, mi, :],
                                     rhs=ef[:, bass.ts(j, QCHUNK)],
                                     start=(mi == 0), stop=(mi == NMC - 1))

            def emit_tail(qc):
                qs = bass.ts(qc, QCHUNK)
                yps = yps_by_qc.pop(qc)
                sums_sb = small.tile([1, QCHUNK], FP32, tag="sums")
                nc.vector.tensor_copy(sums_sb, yps[CI:CI + 1, :])
                rec_sb = small.tile([1, QCHUNK], FP32, tag="rec")
                nc.vector.reciprocal_approx_fast(rec_sb, sums_sb)
                bc = mp.tile([P, QCHUNK], FP32, tag="mp")
                nc.tensor.matmul(bc, lhsT=ones1, rhs=rec_sb,
                                 start=True, stop=True)
                yun_sb = small.tile([CI, QCHUNK], FP32, tag="yun")
                nc.vector.tensor_copy(yun_sb, yps[0:CI, :])
                yn_sb = small.tile([CI, QCHUNK], FP32R, tag="yn")
                nc.vector.tensor_mul(yn_sb, yun_sb, bc[0:CI, :])
                wps = mp.tile([P, QCHUNK], FP32, tag="mp")
                nc.tensor.matmul(wps, lhsT=wo_sb, rhs=yn_sb,
                                 start=True, stop=True)
                nc.vector.bn_stats(stats_sb[:, qc, :], wps)
                nc.vector.tensor_copy(wy_sb[:, qs], wps)

            n_proj = 0

            def need_proj(jreq):
                nonlocal n_proj
                while n_proj <= jreq:
                    emit_proj(n_proj)
                    n_proj += 1

            pending = None
            for qc, gi in stream:
                if qc == 0:
                    need_proj(min(GROUP * gi + GROUP - 1, NMC - 1) // 4)
                ft = emit_ft(qc, gi)
                if pending is not None:
                    pqc, pgi, pft = pending
                    emit_exp_y(pqc, pgi, pft)
                    if pgi == len(groups) - 1:
                        emit_tail(pqc)
                pending = (qc, gi, ft)
            need_proj(7)
            pqc, pgi, pft = pending
            emit_exp_y(pqc, pgi, pft)
            emit_tail(pqc)

            # ---- global BN stats via AllGather ----
            cc_in = dram.tile([P, nqc * 6], FP32)
            nc.sync.dma_start(cc_in,
                              stats_sb[:].rearrange("p a b -> p (a b)"))
            cc_out = dram.tile([NCORES, P, nqc * 6], FP32)
            if model_only:
                for r in range(NCORES):
                    nc.sync.dma_start(cc_out[r], cc_in)
            else:
                nc.gpsimd.collective_compute(
                    "AllGather",
                    mybir.AluOpType.bypass,
                    replica_groups=[list(range(NCORES))],
                    ins=[cc_in[:].opt()],
                    outs=[cc_out[:].opt()],
                )
            gst = big.tile([P, NCORES, nqc * 6], FP32)
            nc.sync.dma_start(gst, cc_out[:].rearrange("r p j -> p r j"))
            gmv = small.tile([P, 2], FP32, tag="gmv")
            nc.vector.bn_aggr(gmv, gst[:].rearrange("p r (a b) -> p (r a) b",
                                                    b=6))
            # rsqrt(var+eps) = exp(-0.5*ln(var+eps)) — stays in the exp/ln
            # ACT table set, avoiding a ~2.7us table switch
            lnv = small.tile([P, 1], FP32, tag="lnv")
            nc.scalar.activation(lnv, gmv[:, 1:2],
                                 mybir.ActivationFunctionType.Ln, bias=epsb)
            inv = small.tile([P, 1], FP32, tag="inv")
            nc.scalar.activation(inv, lnv, Exp, scale=-0.5)
            scl = small.tile([P, 1], FP32, tag="scl")
            nc.vector.tensor_mul(scl, bng_sb, inv)
            msc = small.tile([P, 1], FP32, tag="msc")
            nc.vector.tensor_mul(msc, gmv[:, 0:1], scl)
            sh = small.tile([P, 1], FP32, tag="sh")
            nc.vector.tensor_sub(sh, bnb_sb, msc)

            # ---- apply BN + residual (chunked so DMA overlaps DVE) ----
            out_sb = big.tile([P, NQ], FP32)
            xs_sb = big.tile([P, NQ], FP32)
            Ident = mybir.ActivationFunctionType.Identity
            for j in range(4):
                s = bass.ts(j, 512)
                nc.scalar.activation(xs_sb[:, s], x_sb[:, s].bitcast(FP32),
                                     Ident, bias=sh)
                nc.vector.scalar_tensor_tensor(out_sb[:, s], wy_sb[:, s],
                                               scl, xs_sb[:, s],
                                               op0=mybir.AluOpType.mult,
                                               op1=mybir.AluOpType.add)
                dma_engines[j % 3].dma_start(out[:, s], out_sb[:, s])

    nc.compile()
    return nc


_NC = None


def _get_nc():
    global _NC
    if _NC is None:
        _NC = _build()
    return _NC


def _round_tf32(a):
    """Round-to-nearest-even keeping 11 mantissa bits (fp32r pre-rounding)."""
    u = np.ascontiguousarray(np.asarray(a, np.float32)).view(np.uint32).copy()
    u += np.uint32(0x7FF) + ((u >> np.uint32(12)) & np.uint32(1))
    u &= np.uint32(0xFFFFF000)
    return u.view(np.float32)


def kernel(x, g_w, g_b, theta_w, theta_b, phi_w, phi_b, w_w, w_b,
           bn_gamma, bn_beta, _trace=False):
    x = np.ascontiguousarray(np.asarray(x, dtype=np.float32))
    B = x.shape[0]
    xf = x.reshape(B, P, N)

    f32 = lambda a: np.ascontiguousarray(np.asarray(a, dtype=np.float32))
    common = {
        "wq": _round_tf32(np.asarray(theta_w).T),
        "wk": _round_tf32(np.asarray(phi_w).T),
        "wv": _round_tf32(np.asarray(g_w).T),
        "wo": _round_tf32(np.asarray(w_w).T),
        "tb": f32(np.asarray(theta_b)[:, None]),
        "bng": f32(np.asarray(bn_gamma)[:, None]),
        "bnb": f32(np.asarray(bn_beta)[:, None]),
    }

    in_maps = []
    for core in range(NCORES):
        b, h = core // 2, core % 2
        xs = xf[b]
        xpc = np.concatenate(
            [xs[:, h * NQ:(h + 1) * NQ], xs[:, (1 - h) * NQ:(2 - h) * NQ]],
            axis=1)
        in_maps.append({"xp": _round_tf32(xpc), **common})

    global _last_in_maps
    _last_in_maps = in_maps
    nc = _get_nc()
    res = bass_utils.run_bass_kernel_spmd(nc, in_maps,
                                          core_ids=list(range(NCORES)),
                                          trace=_trace)
    if _trace:
        global last_exec_time_ns, last_results
        last_exec_time_ns = res.exec_time_ns
        last_results = res

    outf = np.empty((B, P, N), dtype=np.float32)
    for core in range(NCORES):
        b, h = core // 2, core % 2
        outf[b, :, h * NQ:(h + 1) * NQ] = res.results[core]["out"]
    return outf.reshape(x.shape)
